# revision 1
# baseline (speedup 1.0000x reference)
"""Trainium2 Bass kernel for nn_Butterfly_1580547970089.

Butterfly multiply (n=1024, log_n=10, nstacks=nblocks=1) + bias over a
16384-row batch, data-parallel across 8 NeuronCores (2048 rows each).

Decomposition (per core, features on partitions, batch on the free dim):
  * Stages 0-6 (strides 1..64) only mix features within 128-blocks; they are
    composed on the host into eight dense 128x128 matrices A_h.
  * Stage 7 (stride 128) pairs adjacent 128-blocks with per-feature 2x2
    coefficients; it is FOLDED into the matmuls: out-tile p of pair (p,q) is
    diag(c00)A_p x_p + diag(c01)A_q x_q, accumulated in PSUM.
  * Each folded matrix is applied with a 3-term bf16 split (W_h x_h + W_h x_e
    + W_e x_h, PSUM accumulates in fp32), giving ~2^-18 relative accuracy at
    bf16 matmul speed. The hi/lo bf16 input planes are prepared on the host,
    so DMA-in bytes equal the fp32 input (8 MB/core).
  * Stages 8 and 9 mix across 128-blocks at strides 256/512 with
    per-partition scalar coefficients: ACT computes the scaled temp
    (activation scale= / bias= per-partition APs, bias of stage 9 fused),
    DVE finishes with scalar_tensor_tensor. Stage 8 reads directly from PSUM
    (doubling as the PSUM evacuation).
  * The feature-major (transposed) layout is produced/consumed on the host as
    part of sharding, so the device does no transposes at all.

Device per core:
  in: xT [2048 rows, 2048 cols] bf16 — row block (ci*1024 + g*128 + p) holds
      feature f=g*128+p of batch chunk ci; cols 0:1024 = hi plane, 1024:2048 = lo.
  weights At [128, 32*128] bf16: per out-tile t: 4 blocks (Bh, Be, Ch, Ce),
      each the TRANSPOSED 128x128 matrix (lhsT layout).
  coef [128, 64] fp32: cols 16..31 s8 coeffs, 32..47 s9, 48..55 bias.
  out: outT [2048, 1024] fp32, same row-block layout, cols = chunk batch.
"""
import numpy as np
import ml_dtypes

import concourse.mybir as mybir
import concourse.tile as tile
from concourse import bacc, bass_utils

F32 = mybir.dt.float32
BF16 = mybir.dt.bfloat16
MULT = mybir.AluOpType.mult
ADD = mybir.AluOpType.add

N_CORES = 8
BATCH = 16384
N = 1024
B_CORE = BATCH // N_CORES
CHUNK = 1024
N_CHUNKS = B_CORE // CHUNK

S7_PAIRS = [(0, 1), (2, 3), (4, 5), (6, 7)]
S8_PAIRS = [(0, 2), (4, 6), (1, 3), (5, 7)]
S9_PAIRS = [(0, 4), (2, 6), (1, 5), (3, 7)]

_compiled = {}


def _emit_kernel(loop_reps=None):
    nc = bacc.Bacc("TRN2", target_bir_lowering=False, debug=False)
    xT = nc.dram_tensor("xT", [N * N_CHUNKS, 2 * CHUNK], BF16,
                        kind="ExternalInput").ap()
    At = nc.dram_tensor("At", [128, 32 * 128], BF16, kind="ExternalInput").ap()
    coef = nc.dram_tensor("coef", [128, 64], F32, kind="ExternalInput").ap()
    outT = nc.dram_tensor("outT", [N * N_CHUNKS, CHUNK], F32,
                          kind="ExternalOutput").ap()

    with tile.TileContext(nc) as tc:
        with (
            tc.tile_pool(name="const", bufs=1) as cpool,
            tc.tile_pool(name="xin", bufs=16) as xpool,
            tc.tile_pool(name="zbuf", bufs=16) as zpool,
            tc.tile_pool(name="tmp", bufs=12) as tpool,
            tc.tile_pool(name="y1", bufs=8, space="PSUM") as ppool,
        ):
            at = cpool.tile([128, 32 * 128], BF16, tag="at")
            nc.sync.dma_start(at[:], At[:])
            cf = cpool.tile([128, 64], F32, tag="cf")
            nc.sync.dma_start(cf[:], coef[:])

            def c(col):
                return cf[:, col:col + 1]

            def w(t, k):
                off = (t * 4 + k) * 128
                return at[:, off:off + 128]

            def chunk_body(ci):
                xt = [None] * 8
                for g in range(8):
                    xt[g] = xpool.tile([128, 2 * CHUNK], BF16, tag="xt",
                                       name=f"xt{g}")
                    row = ci * N + g * 128
                    nc.sync.dma_start(xt[g][:], xT[row:row + 128, :])
                z = [None] * 8
                for g in range(8):
                    z[g] = zpool.tile([128, CHUNK], F32, tag="z", name=f"z{g}")
                for sub in range(0, CHUNK, 512):
                    ps = {}
                    for (p_, q_) in S7_PAIRS:
                        for out_t, main, oth in ((p_, p_, q_), (q_, q_, p_)):
                            pt = ppool.tile([128, 512], F32, tag="y1",
                                            name=f"ps{out_t}")
                            xh_m = xt[main][:, sub:sub + 512]
                            xe_m = xt[main][:, CHUNK + sub:CHUNK + sub + 512]
                            xh_o = xt[oth][:, sub:sub + 512]
                            xe_o = xt[oth][:, CHUNK + sub:CHUNK + sub + 512]
                            nc.tensor.matmul(pt[:], w(out_t, 0), xh_m,
                                             start=True, stop=False)
                            nc.tensor.matmul(pt[:], w(out_t, 0), xe_m,
                                             start=False, stop=False)
                            nc.tensor.matmul(pt[:], w(out_t, 1), xh_m,
                                             start=False, stop=False)
                            nc.tensor.matmul(pt[:], w(out_t, 2), xh_o,
                                             start=False, stop=False)
                            nc.tensor.matmul(pt[:], w(out_t, 2), xe_o,
                                             start=False, stop=False)
                            nc.tensor.matmul(pt[:], w(out_t, 3), xh_o,
                                             start=False, stop=True)
                            ps[out_t] = pt
                    zs = [z[g][:, sub:sub + 512] for g in range(8)]
                    # stage 8: evacuates PSUM
                    for (p_, q_) in S8_PAIRS:
                        base = 16 + 4 * {(0,2):0, (1,3):1, (4,6):2, (5,7):3}[(p_, q_)]
                        u, v = ps[p_], ps[q_]
                        t1 = tpool.tile([128, 512], F32, tag="tmp")
                        nc.scalar.mul(t1[:], v[:], c(base + 1))
                        t2 = tpool.tile([128, 512], F32, tag="tmp")
                        nc.scalar.mul(t2[:], u[:], c(base + 2))
                        nc.vector.scalar_tensor_tensor(
                            zs[p_], u[:], c(base + 0), t1[:], op0=MULT, op1=ADD)
                        nc.vector.scalar_tensor_tensor(
                            zs[q_], v[:], c(base + 3), t2[:], op0=MULT, op1=ADD)
                    # stage 9 with fused bias
                    for (p_, q_) in S9_PAIRS:
                        base = 32 + 4 * {(0,4):0, (1,5):1, (2,6):2, (3,7):3}[(p_, q_)]
                        u, v = zs[p_], zs[q_]
                        t1 = tpool.tile([128, 512], F32, tag="tmp")
                        nc.scalar.activation(
                            t1[:], v, mybir.ActivationFunctionType.Identity,
                            bias=c(48 + p_), scale=c(base + 1))
                        t2 = tpool.tile([128, 512], F32, tag="tmp")
                        nc.scalar.activation(
                            t2[:], u, mybir.ActivationFunctionType.Identity,
                            bias=c(48 + q_), scale=c(base + 2))
                        nc.vector.scalar_tensor_tensor(
                            u, u, c(base + 0), t1[:], op0=MULT, op1=ADD)
                        nc.vector.scalar_tensor_tensor(
                            v, v, c(base + 3), t2[:], op0=MULT, op1=ADD)
                    for g in range(8):
                        row = ci * N + g * 128
                        e = nc.scalar if g % 2 == 0 else nc.sync
                        e.dma_start(
                            outT[row:row + 128, sub:sub + 512],
                            z[g][:, sub:sub + 512])

            def body():
                for ci in range(N_CHUNKS):
                    chunk_body(ci)

            if loop_reps is not None:
                with tc.For_i(0, loop_reps, 1,
                              hint_engines=(mybir.EngineType.PE,
                                            mybir.EngineType.DVE,
                                            mybir.EngineType.Activation)):
                    body()
            else:
                body()

    nc.compile()
    return nc


def _get_compiled(loop_reps=None):
    if loop_reps not in _compiled:
        _compiled[loop_reps] = _emit_kernel(loop_reps)
    return _compiled[loop_reps]


def _build_A(twiddle):
    A = np.zeros((8, 128, 128), np.float64)
    for h in range(8):
        M = np.eye(128, dtype=np.float64)
        for idx in range(7):
            s = 1 << idx
            tw = twiddle[0, 0, idx].astype(np.float64).reshape(512 // s, s, 2, 2)
            tw_h = tw[h * (64 // s):(h + 1) * (64 // s)]
            Mv = M.reshape(64 // s, 2, s, 128)
            top, bot = Mv[:, 0], Mv[:, 1]
            M = np.stack(
                [tw_h[:, :, 0, 0][..., None] * top + tw_h[:, :, 0, 1][..., None] * bot,
                 tw_h[:, :, 1, 0][..., None] * top + tw_h[:, :, 1, 1][..., None] * bot],
                axis=1).reshape(128, 128)
        A[h] = M
    return A


def _split_bf16(M):
    hi = M.astype(ml_dtypes.bfloat16)
    lo = (M - hi.astype(np.float64)).astype(ml_dtypes.bfloat16)
    return hi, lo


def _build_weights(twiddle):
    """At [128, 32*128] bf16: per out-tile t: [Bh, Be, Ch, Ce] transposed."""
    A = _build_A(twiddle)
    t7 = twiddle[0, 0, 7].reshape(4, 128, 2, 2).astype(np.float64)
    At = np.zeros((128, 32 * 128), ml_dtypes.bfloat16)
    for gi, (p, q) in enumerate(S7_PAIRS):
        pairs = [
            (p, np.diag(t7[gi, :, 0, 0]) @ A[p], np.diag(t7[gi, :, 0, 1]) @ A[q]),
            (q, np.diag(t7[gi, :, 1, 1]) @ A[q], np.diag(t7[gi, :, 1, 0]) @ A[p]),
        ]
        for out_t, B, C in pairs:
            Bh, Be = _split_bf16(B.T)
            Ch, Ce = _split_bf16(C.T)
            for k, M in enumerate((Bh, Be, Ch, Ce)):
                off = (out_t * 4 + k) * 128
                At[:, off:off + 128] = M
    return At


def _build_coef(twiddle, bias):
    coef = np.zeros((128, 64), np.float32)
    t8 = twiddle[0, 0, 8].reshape(2, 256, 2, 2)
    t9 = twiddle[0, 0, 9].reshape(1, 512, 2, 2)
    for gi in range(4):
        G, p = divmod(gi, 2)
        sl = slice(p * 128, (p + 1) * 128)
        for k, (i, j) in enumerate([(0, 0), (0, 1), (1, 0), (1, 1)]):
            coef[:, 16 + 4 * gi + k] = t8[G, sl, i, j]
    for p in range(4):
        sl = slice(p * 128, (p + 1) * 128)
        for k, (i, j) in enumerate([(0, 0), (0, 1), (1, 0), (1, 1)]):
            coef[:, 32 + 4 * p + k] = t9[0, sl, i, j]
    coef[:, 48:56] = bias.reshape(8, 128).T
    return coef


def _build_xT(shard):
    """shard [B_CORE, 1024] fp32 -> [N*N_CHUNKS, 2*CHUNK] bf16 blocked layout."""
    out = np.empty((N * N_CHUNKS, 2 * CHUNK), ml_dtypes.bfloat16)
    for ci in range(N_CHUNKS):
        blk = shard[ci * CHUNK:(ci + 1) * CHUNK, :].T  # [1024 f, CHUNK b]
        hi = blk.astype(ml_dtypes.bfloat16)
        lo = (blk - hi.astype(np.float32)).astype(ml_dtypes.bfloat16)
        out[ci * N:(ci + 1) * N, 0:CHUNK] = hi
        out[ci * N:(ci + 1) * N, CHUNK:2 * CHUNK] = lo
    return out


def kernel(input, twiddle, bias):
    input = np.asarray(input)
    twiddle = np.asarray(twiddle)
    bias = np.asarray(bias)
    nc = _get_compiled()

    At = _build_weights(twiddle)
    coef = _build_coef(twiddle, bias)
    in_maps = []
    for cid in range(N_CORES):
        shard = input[cid * B_CORE:(cid + 1) * B_CORE, :]
        in_maps.append({"xT": _build_xT(shard), "At": At, "coef": coef})

    res = bass_utils.run_bass_kernel_spmd(nc, in_maps,
                                          core_ids=list(range(N_CORES)))
    out = np.empty((BATCH, N), np.float32)
    for cid in range(N_CORES):
        o = res.results[cid]["outT"]  # [N*N_CHUNKS, CHUNK]
        for ci in range(N_CHUNKS):
            out[cid * B_CORE + ci * CHUNK:cid * B_CORE + (ci + 1) * CHUNK, :] = \
                o[ci * N:(ci + 1) * N, :].T
    return out



# revision 2
# speedup vs baseline: 1.0300x; 1.0300x over previous
"""Trainium2 Bass kernel for nn_Butterfly_1580547970089.

Butterfly multiply (n=1024, log_n=10, nstacks=nblocks=1) + bias over a
16384-row batch, data-parallel across 8 NeuronCores (2048 rows each).

Decomposition (per core, features on partitions, batch on the free dim):
  * Stages 0-7 are composed on the host into eight pairs of dense 128x128
    bf16 matrices (B_t, C_t): PSUM_t = B_t x_t + C_t x_{other7(t)}.
    The per-feature diagonal of stage 8 and the stage-9 "bake" g_t (the
    diagonal stage-9 coefficient of tile t) are folded into these weights,
    so the vector stages need only multiplicative per-partition ratios.
  * ACT evacuates each PSUM tile to bf16 SBUF (8 ops/sub, the only
    fp32-rate pass).
  * DVE runs everything else in 4x fast mode (all-bf16 SBUF, per-partition
    fp32 scalars are exempt from the 2-byte rule):
      stage 8:  z_t = (u_{P8(t)} * s8_t) + u_t          (scalar_tensor_tensor)
      stage 9:  y_t = (z_{P9(t)} * r9_t) + z_t          (scalar_tensor_tensor)
      bias:     out_t = y_t + b_t                       (tensor_scalar add)
    All scalars are per-feature [128,1] fp32 APs; bias is per-feature so a
    plain per-partition scalar add suffices.
  * Single bf16 plane in (no hi/lo split) and bf16 out: the 2e-2 rel-err
    gate leaves plenty of room (measured ~5e-3), and DMA bytes halve.
  * Few large DMAs (8 in of [128,2048], 24 out, 2 const) keep the serialized
    HWDGE/descriptor overhead (~630 ns per DMA) off the critical path.

Device per core:
  in:  xT  [1024, 2048] bf16 — row g*128+p holds feature f=g*128+p of the
       2048-row batch shard (transposed, single plane).
  At   [128, 16*128] bf16: per out-tile t: (diag(g_t*c8diag_t)@B_t).T then
       (...@C_t).T in lhsT layout.
  coef [128, 24] f32: cols 0-7 s8 ratios, 8-15 s9 ratios, 16-23 bias.
  out: outT [1024, 2048] bf16, same layout as xT.
"""
import numpy as np
import ml_dtypes

import concourse.mybir as mybir
import concourse.tile as tile
from concourse import bacc, bass_utils

F32 = mybir.dt.float32
BF16 = mybir.dt.bfloat16
MULT = mybir.AluOpType.mult
ADD = mybir.AluOpType.add

N_CORES = 8
BATCH = 16384
N = 1024
B_CORE = BATCH // N_CORES   # 2048
CHUNK = B_CORE              # single chunk
N_CHUNKS = 1
SUB = 512
N_SUBS = CHUNK // SUB       # 4

S7_PAIRS = [(0, 1), (2, 3), (4, 5), (6, 7)]
S8_PAIRS = [(0, 2), (1, 3), (4, 6), (5, 7)]
S9_PAIRS = [(0, 4), (1, 5), (2, 6), (3, 7)]
OTHER7 = {0: 1, 1: 0, 2: 3, 3: 2, 4: 5, 5: 4, 6: 7, 7: 6}
P8 = {0: 2, 2: 0, 1: 3, 3: 1, 4: 6, 6: 4, 5: 7, 7: 5}
P9 = {0: 4, 4: 0, 1: 5, 5: 1, 2: 6, 6: 2, 3: 7, 7: 3}

_compiled = {}


def _emit_kernel(loop_reps=None):
    nc = bacc.Bacc("TRN2", target_bir_lowering=False, debug=False)
    xT = nc.dram_tensor("xT", [N, CHUNK], BF16, kind="ExternalInput").ap()
    At = nc.dram_tensor("At", [128, 16 * 128], BF16, kind="ExternalInput").ap()
    coef = nc.dram_tensor("coef", [128, 24], F32, kind="ExternalInput").ap()
    outT = nc.dram_tensor("outT", [N, CHUNK], BF16, kind="ExternalOutput").ap()

    with tile.TileContext(nc) as tc:
        with (
            tc.tile_pool(name="const", bufs=1) as cpool,
            tc.tile_pool(name="xin", bufs=8) as xpool,
            tc.tile_pool(name="uo", bufs=16) as upool,
            tc.tile_pool(name="zo", bufs=16) as zpool,
            tc.tile_pool(name="yo", bufs=16) as ypool,
            tc.tile_pool(name="outb", bufs=8) as opool,
            tc.tile_pool(name="ps", bufs=8, space="PSUM") as ppool,
        ):
            at = cpool.tile([128, 16 * 128], BF16, tag="at")
            nc.sync.dma_start(at[:], At[:])
            cf = cpool.tile([128, 24], F32, tag="cf")
            nc.sync.dma_start(cf[:], coef[:])

            def c(col):
                return cf[:, col:col + 1]

            def w(t, k):
                off = (t * 2 + k) * 128
                return at[:, off:off + 128]

            def body():
                xt = [None] * 8
                for g in range(8):
                    xt[g] = xpool.tile([128, CHUNK], BF16, tag="xt",
                                       name=f"xt{g}")
                    nc.sync.dma_start(xt[g][:], xT[g * 128:(g + 1) * 128, :])
                ot = [None] * 8
                for g in range(8):
                    ot[g] = opool.tile([128, CHUNK], BF16, tag="ot",
                                       name=f"ot{g}")
                for sub in range(N_SUBS):
                    sl = slice(sub * SUB, (sub + 1) * SUB)
                    ps = [None] * 8
                    for t in range(8):
                        ps[t] = ppool.tile([128, SUB], F32, tag="ps",
                                           name=f"ps{t}")
                        nc.tensor.matmul(ps[t][:], w(t, 0), xt[t][:, sl],
                                         start=True, stop=False)
                        nc.tensor.matmul(ps[t][:], w(t, 1),
                                         xt[OTHER7[t]][:, sl],
                                         start=False, stop=True)
                    u = [None] * 8
                    for t in range(8):
                        u[t] = upool.tile([128, SUB], BF16, tag="u",
                                          name=f"u{t}")
                        nc.scalar.copy(u[t][:], ps[t][:])
                    z = [None] * 8
                    for t in range(8):
                        z[t] = zpool.tile([128, SUB], BF16, tag="z",
                                          name=f"z{t}")
                        nc.vector.scalar_tensor_tensor(
                            z[t][:], u[P8[t]][:], c(t), u[t][:],
                            op0=MULT, op1=ADD)
                    y = [None] * 8
                    for t in range(8):
                        y[t] = ypool.tile([128, SUB], BF16, tag="y",
                                          name=f"y{t}")
                        nc.vector.scalar_tensor_tensor(
                            y[t][:], z[P9[t]][:], c(8 + t), z[t][:],
                            op0=MULT, op1=ADD)
                    for t in range(8):
                        nc.vector.tensor_scalar(
                            ot[t][:, sl], y[t][:], c(16 + t), None, op0=ADD)
                    # staggered output DMA: [0:1024] after sub 1, then
                    # [1024:1536], [1536:2048] per-sub to shorten the tail
                    if sub >= 1:
                        lo = 0 if sub == 1 else sub * SUB
                        osl = slice(lo, (sub + 1) * SUB)
                        for t in range(8):
                            nc.sync.dma_start(
                                outT[t * 128:(t + 1) * 128, osl],
                                ot[t][:, osl])

            if loop_reps is not None:
                with tc.For_i(0, loop_reps, 1,
                              hint_engines=(mybir.EngineType.PE,
                                            mybir.EngineType.DVE,
                                            mybir.EngineType.Activation)):
                    body()
            else:
                body()

    nc.compile()
    return nc


def _get_compiled(loop_reps=None):
    if loop_reps not in _compiled:
        _compiled[loop_reps] = _emit_kernel(loop_reps)
    return _compiled[loop_reps]


def _build_A(twiddle):
    A = np.zeros((8, 128, 128), np.float64)
    for h in range(8):
        M = np.eye(128, dtype=np.float64)
        for idx in range(7):
            s = 1 << idx
            tw = twiddle[0, 0, idx].astype(np.float64).reshape(512 // s, s, 2, 2)
            tw_h = tw[h * (64 // s):(h + 1) * (64 // s)]
            Mv = M.reshape(64 // s, 2, s, 128)
            top, bot = Mv[:, 0], Mv[:, 1]
            M = np.stack(
                [tw_h[:, :, 0, 0][..., None] * top + tw_h[:, :, 0, 1][..., None] * bot,
                 tw_h[:, :, 1, 0][..., None] * top + tw_h[:, :, 1, 1][..., None] * bot],
                axis=1).reshape(128, 128)
        A[h] = M
    return A


def _coef_parts(twiddle):
    t8 = twiddle[0, 0, 8].reshape(2, 256, 2, 2).astype(np.float64)
    t9 = twiddle[0, 0, 9].reshape(512, 2, 2).astype(np.float64)
    c8d = np.zeros((8, 128)); c8o = np.zeros((8, 128))
    for gi, (p_, q_) in enumerate(S8_PAIRS):
        G, hp = divmod(gi, 2)
        cc = t8[G, hp * 128:(hp + 1) * 128]
        c8d[p_], c8o[p_] = cc[:, 0, 0], cc[:, 0, 1]
        c8d[q_], c8o[q_] = cc[:, 1, 1], cc[:, 1, 0]
    g = np.zeros((8, 128)); r9 = np.zeros((8, 128))
    for a, b in S9_PAIRS:
        e = t9[a * 128:(a + 1) * 128]
        g[a], g[b] = e[:, 0, 0], e[:, 1, 1]
        r9[a] = e[:, 0, 1] / e[:, 1, 1]
        r9[b] = e[:, 1, 0] / e[:, 0, 0]
    alpha = g * c8d
    s8 = np.zeros((8, 128))
    for t in range(8):
        s8[t] = g[t] * c8o[t] / alpha[P8[t]]
    return alpha, s8, r9


def _build_weights(twiddle):
    """At [128, 16*128] bf16: per out-tile t: (diag(alpha_t)B_t).T, (..C_t).T."""
    A = _build_A(twiddle)
    t7 = twiddle[0, 0, 7].reshape(4, 128, 2, 2).astype(np.float64)
    B = np.zeros((8, 128, 128)); C = np.zeros((8, 128, 128))
    for gi, (p, q) in enumerate(S7_PAIRS):
        B[p] = np.diag(t7[gi, :, 0, 0]) @ A[p]
        C[p] = np.diag(t7[gi, :, 0, 1]) @ A[q]
        B[q] = np.diag(t7[gi, :, 1, 1]) @ A[q]
        C[q] = np.diag(t7[gi, :, 1, 0]) @ A[p]
    alpha, _, _ = _coef_parts(twiddle)
    At = np.zeros((128, 16 * 128), ml_dtypes.bfloat16)
    for t in range(8):
        Wb = (alpha[t][:, None] * B[t]).T
        Wc = (alpha[t][:, None] * C[t]).T
        At[:, (t * 2) * 128:(t * 2 + 1) * 128] = Wb.astype(ml_dtypes.bfloat16)
        At[:, (t * 2 + 1) * 128:(t * 2 + 2) * 128] = Wc.astype(ml_dtypes.bfloat16)
    return At


def _build_coef(twiddle, bias):
    _, s8, r9 = _coef_parts(twiddle)
    coef = np.zeros((128, 24), np.float32)
    coef[:, 0:8] = s8.T
    coef[:, 8:16] = r9.T
    coef[:, 16:24] = np.asarray(bias, np.float64).reshape(8, 128).T
    return coef


def _build_xT(shard):
    """shard [B_CORE, 1024] fp32 -> [1024, B_CORE] bf16 (transposed)."""
    return np.ascontiguousarray(shard.T).astype(ml_dtypes.bfloat16)


def kernel(input, twiddle, bias):
    input = np.asarray(input)
    twiddle = np.asarray(twiddle)
    bias = np.asarray(bias)
    nc = _get_compiled()

    At = _build_weights(twiddle)
    coef = _build_coef(twiddle, bias)
    in_maps = []
    for cid in range(N_CORES):
        shard = input[cid * B_CORE:(cid + 1) * B_CORE, :]
        in_maps.append({"xT": _build_xT(shard), "At": At, "coef": coef})

    res = bass_utils.run_bass_kernel_spmd(nc, in_maps,
                                          core_ids=list(range(N_CORES)))
    out = np.empty((BATCH, N), np.float32)
    for cid in range(N_CORES):
        o = res.results[cid]["outT"]  # [1024, B_CORE] bf16
        out[cid * B_CORE:(cid + 1) * B_CORE, :] = o.T.astype(np.float32)
    return out


# revision 3
# speedup vs baseline: 1.4275x; 1.3859x over previous
"""Trainium2 Bass kernel for nn_Butterfly_1580547970089.

Butterfly multiply (n=1024, log_n=10, nstacks=nblocks=1) + bias over a
16384-row batch, data-parallel across 8 NeuronCores (2048 rows each).

Decomposition (per core, features on partitions, batch on the free dim):
  * Stages 0-8 are composed on the host into four dense 128x128 bf16
    matrices per out-tile (stage 8 mixes tile t with P8(t), so out-tile t
    draws on 4 input tiles: t, other7(t), P8(t), other7(P8(t))). The
    per-feature stage-9 "bake" g_t (tile t's diagonal stage-9 coefficient)
    is also folded into the weights. PSUM_t = sum_k W(t,k) x_{src_k}.
  * ACT evacuates each PSUM tile to bf16 SBUF (the only fp32-rate pass).
  * DVE finishes stage 9 with ops that support the 4x/2x fast modes
    (scalar_tensor_tensor does NOT; tensor_scalar and tensor_tensor do):
      prescale: v_t = (u_{P9(t)} * r9_t) + b_t     (tensor_scalar, 2 scalars,
                4x mode, bias folded — scalars are per-partition fp32 APs)
      combine:  out_t = u_t + v_t                  (tensor_tensor, 2x mode)
    All per-feature coefficients are [128,1] fp32 APs (features live on
    partitions). Ratios are purely multiplicative — numerically safe.
  * Single bf16 plane in (no hi/lo split) and bf16 out: the 2e-2 rel-err
    gate leaves plenty of room (~5e-3 measured), and DMA bytes halve.
  * Few large DMAs (8 in of [128,2048], 24 staggered out, 2 const) keep the
    serialized per-DMA HWDGE overhead (~630 ns) off the critical path.

Device per core:
  in:  xT  [1024, 2048] bf16 — row g*128+p holds feature f=g*128+p of the
       2048-row batch shard (transposed, single plane).
  At   [128, 32*128] bf16: per out-tile t: 4 blocks W(t,0..3).T (lhsT).
  coef [128, 16] f32: cols 0-7 r9 ratios, 8-15 bias.
  out: outT [1024, 2048] bf16, same layout as xT.
"""
import numpy as np
import ml_dtypes

import concourse.mybir as mybir
import concourse.tile as tile
from concourse import bacc, bass_utils

F32 = mybir.dt.float32
BF16 = mybir.dt.bfloat16
MULT = mybir.AluOpType.mult
ADD = mybir.AluOpType.add

N_CORES = 8
BATCH = 16384
N = 1024
B_CORE = BATCH // N_CORES   # 2048
CHUNK = B_CORE              # single chunk
N_CHUNKS = 1
SUB = 512
N_SUBS = CHUNK // SUB       # 4

S7_PAIRS = [(0, 1), (2, 3), (4, 5), (6, 7)]
S8_PAIRS = [(0, 2), (1, 3), (4, 6), (5, 7)]
S9_PAIRS = [(0, 4), (1, 5), (2, 6), (3, 7)]
OTHER7 = {0: 1, 1: 0, 2: 3, 3: 2, 4: 5, 5: 4, 6: 7, 7: 6}
P8 = {0: 2, 2: 0, 1: 3, 3: 1, 4: 6, 6: 4, 5: 7, 7: 5}
P9 = {0: 4, 4: 0, 1: 5, 5: 1, 2: 6, 6: 2, 3: 7, 7: 3}
SRC = {t: (t, OTHER7[t], P8[t], OTHER7[P8[t]]) for t in range(8)}

_compiled = {}


def _emit_kernel(loop_reps=None):
    nc = bacc.Bacc("TRN2", target_bir_lowering=False, debug=False)
    xT = nc.dram_tensor("xT", [N, CHUNK], BF16, kind="ExternalInput").ap()
    At = nc.dram_tensor("At", [128, 32 * 128], BF16, kind="ExternalInput").ap()
    coef = nc.dram_tensor("coef", [128, 16], F32, kind="ExternalInput").ap()
    outT = nc.dram_tensor("outT", [N, CHUNK], BF16, kind="ExternalOutput").ap()

    with tile.TileContext(nc) as tc:
        with (
            tc.tile_pool(name="const", bufs=1) as cpool,
            tc.tile_pool(name="xin", bufs=8) as xpool,
            tc.tile_pool(name="uo", bufs=16) as upool,
            tc.tile_pool(name="vo", bufs=16) as vpool,
            tc.tile_pool(name="outb", bufs=8) as opool,
            tc.tile_pool(name="ps", bufs=8, space="PSUM") as ppool,
        ):
            at = cpool.tile([128, 32 * 128], BF16, tag="at")
            nc.sync.dma_start(at[:], At[:])
            cf = cpool.tile([128, 16], F32, tag="cf")
            nc.sync.dma_start(cf[:], coef[:])

            def c(col):
                return cf[:, col:col + 1]

            def w(t, k):
                off = (t * 4 + k) * 128
                return at[:, off:off + 128]

            def body():
                xt = [None] * 8
                for g in range(8):
                    xt[g] = xpool.tile([128, CHUNK], BF16, tag="xt",
                                       name=f"xt{g}")
                    nc.sync.dma_start(xt[g][:], xT[g * 128:(g + 1) * 128, :])
                ot = [None] * 8
                for g in range(8):
                    ot[g] = opool.tile([128, CHUNK], BF16, tag="ot",
                                       name=f"ot{g}")
                for sub in range(N_SUBS):
                    sl = slice(sub * SUB, (sub + 1) * SUB)
                    ps = [None] * 8
                    for t in range(8):
                        ps[t] = ppool.tile([128, SUB], F32, tag="ps",
                                           name=f"ps{t}")
                        for k in range(4):
                            nc.tensor.matmul(ps[t][:], w(t, k),
                                             xt[SRC[t][k]][:, sl],
                                             start=(k == 0), stop=(k == 3))
                    u = [None] * 8
                    for t in range(8):
                        u[t] = upool.tile([128, SUB], BF16, tag="u",
                                          name=f"u{t}")
                        nc.scalar.copy(u[t][:], ps[t][:])
                    v = [None] * 8
                    for t in range(8):
                        v[t] = vpool.tile([128, SUB], BF16, tag="v",
                                          name=f"v{t}")
                        nc.vector.tensor_scalar(v[t][:], u[P9[t]][:],
                                                c(t), c(8 + t),
                                                op0=MULT, op1=ADD)
                    for t in range(8):
                        nc.vector.tensor_tensor(ot[t][:, sl], u[t][:],
                                                v[t][:], op=ADD)
                    # staggered output DMA: [0:1024] after sub 1, then
                    # [1024:1536], [1536:2048] per-sub to shorten the tail
                    if sub >= 1:
                        lo = 0 if sub == 1 else sub * SUB
                        osl = slice(lo, (sub + 1) * SUB)
                        for t in range(8):
                            nc.sync.dma_start(
                                outT[t * 128:(t + 1) * 128, osl],
                                ot[t][:, osl])

            if loop_reps is not None:
                with tc.For_i(0, loop_reps, 1,
                              hint_engines=(mybir.EngineType.PE,
                                            mybir.EngineType.DVE,
                                            mybir.EngineType.Activation)):
                    body()
            else:
                body()

    nc.compile()
    return nc


def _get_compiled(loop_reps=None):
    if loop_reps not in _compiled:
        _compiled[loop_reps] = _emit_kernel(loop_reps)
    return _compiled[loop_reps]


def _build_A(twiddle):
    A = np.zeros((8, 128, 128), np.float64)
    for h in range(8):
        M = np.eye(128, dtype=np.float64)
        for idx in range(7):
            s = 1 << idx
            tw = twiddle[0, 0, idx].astype(np.float64).reshape(512 // s, s, 2, 2)
            tw_h = tw[h * (64 // s):(h + 1) * (64 // s)]
            Mv = M.reshape(64 // s, 2, s, 128)
            top, bot = Mv[:, 0], Mv[:, 1]
            M = np.stack(
                [tw_h[:, :, 0, 0][..., None] * top + tw_h[:, :, 0, 1][..., None] * bot,
                 tw_h[:, :, 1, 0][..., None] * top + tw_h[:, :, 1, 1][..., None] * bot],
                axis=1).reshape(128, 128)
        A[h] = M
    return A


def _coef_parts(twiddle):
    t8 = twiddle[0, 0, 8].reshape(2, 256, 2, 2).astype(np.float64)
    t9 = twiddle[0, 0, 9].reshape(512, 2, 2).astype(np.float64)
    c8d = np.zeros((8, 128)); c8o = np.zeros((8, 128))
    for gi, (p_, q_) in enumerate(S8_PAIRS):
        G, hp = divmod(gi, 2)
        cc = t8[G, hp * 128:(hp + 1) * 128]
        c8d[p_], c8o[p_] = cc[:, 0, 0], cc[:, 0, 1]
        c8d[q_], c8o[q_] = cc[:, 1, 1], cc[:, 1, 0]
    g = np.zeros((8, 128)); r9 = np.zeros((8, 128))
    for a, b in S9_PAIRS:
        e = t9[a * 128:(a + 1) * 128]
        g[a], g[b] = e[:, 0, 0], e[:, 1, 1]
        r9[a] = e[:, 0, 1] / e[:, 1, 1]
        r9[b] = e[:, 1, 0] / e[:, 0, 0]
    return c8d, c8o, g, r9


def _build_weights(twiddle):
    """At [128, 32*128] bf16: per out-tile t: W(t,0..3).T per SRC order."""
    A = _build_A(twiddle)
    t7 = twiddle[0, 0, 7].reshape(4, 128, 2, 2).astype(np.float64)
    B = np.zeros((8, 128, 128)); C = np.zeros((8, 128, 128))
    for gi, (p, q) in enumerate(S7_PAIRS):
        B[p] = np.diag(t7[gi, :, 0, 0]) @ A[p]
        C[p] = np.diag(t7[gi, :, 0, 1]) @ A[q]
        B[q] = np.diag(t7[gi, :, 1, 1]) @ A[q]
        C[q] = np.diag(t7[gi, :, 1, 0]) @ A[p]
    c8d, c8o, g, _ = _coef_parts(twiddle)
    At = np.zeros((128, 32 * 128), ml_dtypes.bfloat16)
    for t in range(8):
        pt = P8[t]
        # W applied to x_src for src = t, o7(t), P8(t), o7(P8(t))
        Ws = [
            (g[t] * c8d[t])[:, None] * B[t],
            (g[t] * c8d[t])[:, None] * C[t],
            (g[t] * c8o[t])[:, None] * B[pt],
            (g[t] * c8o[t])[:, None] * C[pt],
        ]
        for k, W in enumerate(Ws):
            off = (t * 4 + k) * 128
            At[:, off:off + 128] = W.T.astype(ml_dtypes.bfloat16)
    return At


def _build_coef(twiddle, bias):
    _, _, _, r9 = _coef_parts(twiddle)
    coef = np.zeros((128, 16), np.float32)
    coef[:, 0:8] = r9.T
    coef[:, 8:16] = np.asarray(bias, np.float64).reshape(8, 128).T
    return coef


def _build_xT(shard):
    """shard [B_CORE, 1024] fp32 -> [1024, B_CORE] bf16 (transposed)."""
    return np.ascontiguousarray(shard.T).astype(ml_dtypes.bfloat16)


def kernel(input, twiddle, bias):
    input = np.asarray(input)
    twiddle = np.asarray(twiddle)
    bias = np.asarray(bias)
    nc = _get_compiled()

    At = _build_weights(twiddle)
    coef = _build_coef(twiddle, bias)
    in_maps = []
    for cid in range(N_CORES):
        shard = input[cid * B_CORE:(cid + 1) * B_CORE, :]
        in_maps.append({"xT": _build_xT(shard), "At": At, "coef": coef})

    res = bass_utils.run_bass_kernel_spmd(nc, in_maps,
                                          core_ids=list(range(N_CORES)))
    out = np.empty((BATCH, N), np.float32)
    for cid in range(N_CORES):
        o = res.results[cid]["outT"]  # [1024, B_CORE] bf16
        out[cid * B_CORE:(cid + 1) * B_CORE, :] = o.T.astype(np.float32)
    return out


# revision 4
# speedup vs baseline: 1.4461x; 1.0131x over previous
"""Trainium2 Bass kernel for nn_Butterfly_1580547970089.

Butterfly multiply (n=1024, log_n=10, nstacks=nblocks=1) + bias over a
16384-row batch, data-parallel across 8 NeuronCores (2048 rows each).

Decomposition (per core, features on partitions, batch on the free dim):
  * Stages 0-7 are composed on the host into dense 128x128 bf16 matrices.
    For out-tiles 0-3 stage 8 is ALSO folded into the weights (4 matmuls
    from sources {t, other7(t), P8(t), other7(P8(t))}); out-tiles 4-7 use
    2 matmuls and do stage 8 on DVE. This splits the load: PE ~20 us,
    DVE ~23 us, ACT ~23 us, DMA bus ~26 us (the floor).
  * The per-feature stage-9 bake g_t and tile-diagonal stage-8 coefficient
    are folded into the matmul weights, so vector stages use only fast-mode
    ops (scalar_tensor_tensor has NO DVE fast mode; tensor_scalar 4x and
    tensor_tensor 2x do):
      evac:    u_t = bf16(PSUM_t)                      (ACT, the fp32 pass)
      stage 8: v8_t = u_{P8(t)} * s8_t                 (ts 1-scalar, 4x)
               z_t  = u_t + v8_t                       (tt, 2x)   tiles 4-7
      stage 9: v9_t = y_{P9(t)} * r9_t + b_t           (ts2 on DVE 4x /
               out_t = y_t + v9_t                       ACT activation; tt 2x)
    where y_t = u_t (folded tiles) or z_t (vector tiles). Scalars are
    per-partition [128,1] fp32 APs; ratios are multiplicative (safe), bias
    adds at its natural place.
  * Single bf16 plane in + bf16 out halves DMA bytes (rel err ~5e-3 vs the
    2e-2 gate).
  * Input streams in [128,1024] halves so compute starts ~1.5 us in; output
    DMAs go through gpsimd (SWDGE) to keep the serialized HWDGE overhead
    (~630 ns/DMA) off the input path.

Device per core:
  in:  xT  [1024, 2048] bf16 — row g*128+p = feature, col = batch in shard.
  At   [128, 24*128] bf16: tiles 0-3: 4 lhsT blocks each; tiles 4-7: 2 each.
  coef [128, 24] f32: 0-7 r9, 8-15 bias, 16-19 s8 (tiles 4-7).
  out: outT [1024, 2048] bf16, same layout as xT.
"""
import numpy as np
import ml_dtypes

import concourse.mybir as mybir
import concourse.tile as tile
from concourse import bacc, bass_utils

F32 = mybir.dt.float32
BF16 = mybir.dt.bfloat16
MULT = mybir.AluOpType.mult
ADD = mybir.AluOpType.add

N_CORES = 8
BATCH = 16384
N = 1024
B_CORE = BATCH // N_CORES   # 2048
CHUNK = B_CORE              # single chunk
N_CHUNKS = 1
SUB = 512
N_SUBS = CHUNK // SUB       # 4

S7_PAIRS = [(0, 1), (2, 3), (4, 5), (6, 7)]
S8_PAIRS = [(0, 2), (1, 3), (4, 6), (5, 7)]
S9_PAIRS = [(0, 4), (1, 5), (2, 6), (3, 7)]
OTHER7 = {0: 1, 1: 0, 2: 3, 3: 2, 4: 5, 5: 4, 6: 7, 7: 6}
P8 = {0: 2, 2: 0, 1: 3, 3: 1, 4: 6, 6: 4, 5: 7, 7: 5}
P9 = {0: 4, 4: 0, 1: 5, 5: 1, 2: 6, 6: 2, 3: 7, 7: 3}
FOLDED = (0, 1, 2, 3)       # stage 8 folded into PE
VECT = (4, 5, 6, 7)         # stage 8 on DVE
SRC = {t: (t, OTHER7[t], P8[t], OTHER7[P8[t]]) for t in FOLDED}

_compiled = {}


def _woff(t, k):
    if t < 4:
        return (t * 4 + k) * 128
    return (16 + (t - 4) * 2 + k) * 128


def _emit_kernel(loop_reps=None):
    nc = bacc.Bacc("TRN2", target_bir_lowering=False, debug=False)
    xT = nc.dram_tensor("xT", [N, CHUNK], BF16, kind="ExternalInput").ap()
    At = nc.dram_tensor("At", [128, 24 * 128], BF16, kind="ExternalInput").ap()
    coef = nc.dram_tensor("coef", [128, 24], F32, kind="ExternalInput").ap()
    outT = nc.dram_tensor("outT", [N, CHUNK], BF16, kind="ExternalOutput").ap()

    with tile.TileContext(nc) as tc:
        with (
            tc.tile_pool(name="const", bufs=1) as cpool,
            tc.tile_pool(name="xin", bufs=8) as xpool,
            tc.tile_pool(name="uo", bufs=16) as upool,
            tc.tile_pool(name="v8o", bufs=8) as v8pool,
            tc.tile_pool(name="zo", bufs=8) as zpool,
            tc.tile_pool(name="v9o", bufs=16) as v9pool,
            tc.tile_pool(name="outb", bufs=8) as opool,
            tc.tile_pool(name="ps", bufs=8, space="PSUM") as ppool,
        ):
            at = cpool.tile([128, 24 * 128], BF16, tag="at")
            nc.sync.dma_start(at[:], At[:])
            cf = cpool.tile([128, 24], F32, tag="cf")
            nc.sync.dma_start(cf[:], coef[:])

            def c(col):
                return cf[:, col:col + 1]

            def w(t, k):
                off = _woff(t, k)
                return at[:, off:off + 128]

            def body():
                xt = [None] * 8
                for g in range(8):
                    xt[g] = xpool.tile([128, CHUNK], BF16, tag="xt",
                                       name=f"xt{g}")
                # stream input in halves so sub-0 compute starts early
                for half in range(2):
                    hs = slice(half * 1024, (half + 1) * 1024)
                    for g in range(8):
                        nc.sync.dma_start(xt[g][:, hs],
                                          xT[g * 128:(g + 1) * 128, hs])
                ot = [None] * 8
                for g in range(8):
                    ot[g] = opool.tile([128, CHUNK], BF16, tag="ot",
                                       name=f"ot{g}")
                for sub in range(N_SUBS):
                    sl = slice(sub * SUB, (sub + 1) * SUB)
                    ps = [None] * 8
                    for t in FOLDED:
                        ps[t] = ppool.tile([128, SUB], F32, tag="ps",
                                           name=f"ps{t}")
                        for k in range(4):
                            nc.tensor.matmul(ps[t][:], w(t, k),
                                             xt[SRC[t][k]][:, sl],
                                             start=(k == 0), stop=(k == 3))
                    for t in VECT:
                        ps[t] = ppool.tile([128, SUB], F32, tag="ps",
                                           name=f"ps{t}")
                        nc.tensor.matmul(ps[t][:], w(t, 0), xt[t][:, sl],
                                         start=True, stop=False)
                        nc.tensor.matmul(ps[t][:], w(t, 1),
                                         xt[OTHER7[t]][:, sl],
                                         start=False, stop=True)
                    u = [None] * 8
                    for t in range(8):
                        u[t] = upool.tile([128, SUB], BF16, tag="u",
                                          name=f"u{t}")
                        nc.scalar.copy(u[t][:], ps[t][:])
                    # stage 8 for tiles 4-7 on DVE
                    y = list(u)
                    v8 = {}
                    for t in VECT:
                        v8[t] = v8pool.tile([128, SUB], BF16, tag="v8",
                                            name=f"v8{t}")
                        nc.vector.tensor_scalar(v8[t][:], u[P8[t]][:],
                                                c(16 + t - 4), None, op0=MULT)
                    for t in VECT:
                        z = zpool.tile([128, SUB], BF16, tag="z",
                                       name=f"z{t}")
                        nc.vector.tensor_tensor(z[:], u[t][:], v8[t][:],
                                                op=ADD)
                        y[t] = z
                    # stage 9: prescale+bias then combine
                    v9 = [None] * 8
                    for t in range(8):
                        v9[t] = v9pool.tile([128, SUB], BF16, tag="v9",
                                            name=f"v9{t}")
                        if t < 2:   # offload two prescales to ACT
                            nc.scalar.activation(
                                v9[t][:], y[P9[t]][:],
                                mybir.ActivationFunctionType.Identity,
                                bias=c(8 + t), scale=c(t))
                        else:
                            nc.vector.tensor_scalar(v9[t][:], y[P9[t]][:],
                                                    c(t), c(8 + t),
                                                    op0=MULT, op1=ADD)
                    for t in range(8):
                        nc.vector.tensor_tensor(ot[t][:, sl], y[t][:],
                                                v9[t][:], op=ADD)
                    # staggered output DMA via SWDGE (gpsimd): [0:1024]
                    # after sub 1, then [1024:1536], [1536:2048]
                    if sub >= 1:
                        lo = 0 if sub == 1 else sub * SUB
                        osl = slice(lo, (sub + 1) * SUB)
                        for t in range(8):
                            nc.gpsimd.dma_start(
                                outT[t * 128:(t + 1) * 128, osl],
                                ot[t][:, osl])

            if loop_reps is not None:
                with tc.For_i(0, loop_reps, 1,
                              hint_engines=(mybir.EngineType.PE,
                                            mybir.EngineType.DVE,
                                            mybir.EngineType.Activation)):
                    body()
            else:
                body()

    nc.compile()
    return nc


def _get_compiled(loop_reps=None):
    if loop_reps not in _compiled:
        _compiled[loop_reps] = _emit_kernel(loop_reps)
    return _compiled[loop_reps]


def _build_A(twiddle):
    A = np.zeros((8, 128, 128), np.float64)
    for h in range(8):
        M = np.eye(128, dtype=np.float64)
        for idx in range(7):
            s = 1 << idx
            tw = twiddle[0, 0, idx].astype(np.float64).reshape(512 // s, s, 2, 2)
            tw_h = tw[h * (64 // s):(h + 1) * (64 // s)]
            Mv = M.reshape(64 // s, 2, s, 128)
            top, bot = Mv[:, 0], Mv[:, 1]
            M = np.stack(
                [tw_h[:, :, 0, 0][..., None] * top + tw_h[:, :, 0, 1][..., None] * bot,
                 tw_h[:, :, 1, 0][..., None] * top + tw_h[:, :, 1, 1][..., None] * bot],
                axis=1).reshape(128, 128)
        A[h] = M
    return A


def _coef_parts(twiddle):
    t8 = twiddle[0, 0, 8].reshape(2, 256, 2, 2).astype(np.float64)
    t9 = twiddle[0, 0, 9].reshape(512, 2, 2).astype(np.float64)
    c8d = np.zeros((8, 128)); c8o = np.zeros((8, 128))
    for gi, (p_, q_) in enumerate(S8_PAIRS):
        G, hp = divmod(gi, 2)
        cc = t8[G, hp * 128:(hp + 1) * 128]
        c8d[p_], c8o[p_] = cc[:, 0, 0], cc[:, 0, 1]
        c8d[q_], c8o[q_] = cc[:, 1, 1], cc[:, 1, 0]
    g = np.zeros((8, 128)); r9 = np.zeros((8, 128))
    for a, b in S9_PAIRS:
        e = t9[a * 128:(a + 1) * 128]
        g[a], g[b] = e[:, 0, 0], e[:, 1, 1]
        r9[a] = e[:, 0, 1] / e[:, 1, 1]
        r9[b] = e[:, 1, 0] / e[:, 0, 0]
    return c8d, c8o, g, r9


def _build_weights(twiddle):
    """At [128, 24*128] bf16 per _woff layout (lhsT blocks)."""
    A = _build_A(twiddle)
    t7 = twiddle[0, 0, 7].reshape(4, 128, 2, 2).astype(np.float64)
    B = np.zeros((8, 128, 128)); C = np.zeros((8, 128, 128))
    for gi, (p, q) in enumerate(S7_PAIRS):
        B[p] = np.diag(t7[gi, :, 0, 0]) @ A[p]
        C[p] = np.diag(t7[gi, :, 0, 1]) @ A[q]
        B[q] = np.diag(t7[gi, :, 1, 1]) @ A[q]
        C[q] = np.diag(t7[gi, :, 1, 0]) @ A[p]
    c8d, c8o, g, _ = _coef_parts(twiddle)
    At = np.zeros((128, 24 * 128), ml_dtypes.bfloat16)
    for t in FOLDED:
        pt = P8[t]
        Ws = [
            (g[t] * c8d[t])[:, None] * B[t],
            (g[t] * c8d[t])[:, None] * C[t],
            (g[t] * c8o[t])[:, None] * B[pt],
            (g[t] * c8o[t])[:, None] * C[pt],
        ]
        for k, W in enumerate(Ws):
            off = _woff(t, k)
            At[:, off:off + 128] = W.T.astype(ml_dtypes.bfloat16)
    for t in VECT:
        alpha = g[t] * c8d[t]
        for k, W in enumerate((alpha[:, None] * B[t], alpha[:, None] * C[t])):
            off = _woff(t, k)
            At[:, off:off + 128] = W.T.astype(ml_dtypes.bfloat16)
    return At


def _build_coef(twiddle, bias):
    c8d, c8o, g, r9 = _coef_parts(twiddle)
    coef = np.zeros((128, 24), np.float32)
    coef[:, 0:8] = r9.T
    coef[:, 8:16] = np.asarray(bias, np.float64).reshape(8, 128).T
    for t in VECT:
        alpha_p = g[P8[t]] * c8d[P8[t]]
        coef[:, 16 + t - 4] = g[t] * c8o[t] / alpha_p
    return coef


def _build_xT(shard):
    """shard [B_CORE, 1024] fp32 -> [1024, B_CORE] bf16 (transposed)."""
    return np.ascontiguousarray(shard.T).astype(ml_dtypes.bfloat16)


def kernel(input, twiddle, bias):
    input = np.asarray(input)
    twiddle = np.asarray(twiddle)
    bias = np.asarray(bias)
    nc = _get_compiled()

    At = _build_weights(twiddle)
    coef = _build_coef(twiddle, bias)
    in_maps = []
    for cid in range(N_CORES):
        shard = input[cid * B_CORE:(cid + 1) * B_CORE, :]
        in_maps.append({"xT": _build_xT(shard), "At": At, "coef": coef})

    res = bass_utils.run_bass_kernel_spmd(nc, in_maps,
                                          core_ids=list(range(N_CORES)))
    out = np.empty((BATCH, N), np.float32)
    for cid in range(N_CORES):
        o = res.results[cid]["outT"]  # [1024, B_CORE] bf16
        out[cid * B_CORE:(cid + 1) * B_CORE, :] = o.T.astype(np.float32)
    return out


# revision 5
# speedup vs baseline: 1.4943x; 1.0333x over previous
"""Trainium2 Bass kernel for nn_Butterfly_1580547970089.

Butterfly multiply (n=1024, log_n=10, nstacks=nblocks=1) + bias over a
16384-row batch, data-parallel across 8 NeuronCores (2048 rows each).

Decomposition (per core, features on partitions, batch on the free dim):
  * Stages 0-7 are composed on the host into dense 128x128 bf16 matrices.
    For out-tiles 0-3 stage 8 is ALSO folded into the weights (4 matmuls
    from sources {t, other7(t), P8(t), other7(P8(t))}); out-tiles 4-7 use
    2 matmuls and do stage 8 on DVE. This splits the load: PE ~20 us,
    DVE ~23 us, ACT ~23 us, DMA bus ~26 us (the floor).
  * The per-feature stage-9 bake g_t and tile-diagonal stage-8 coefficient
    are folded into the matmul weights, so vector stages use only fast-mode
    ops (scalar_tensor_tensor has NO DVE fast mode; tensor_scalar 4x and
    tensor_tensor 2x do):
      evac:    u_t = bf16(PSUM_t)                      (ACT, the fp32 pass)
      stage 8: v8_t = u_{P8(t)} * s8_t                 (ts 1-scalar, 4x)
               z_t  = u_t + v8_t                       (tt, 2x)   tiles 4-7
      stage 9: v9_t = y_{P9(t)} * r9_t + b_t           (ts2 on DVE 4x /
               out_t = y_t + v9_t                       ACT activation; tt 2x)
    where y_t = u_t (folded tiles) or z_t (vector tiles). Scalars are
    per-partition [128,1] fp32 APs; ratios are multiplicative (safe), bias
    adds at its natural place.
  * Single bf16 plane in + bf16 out halves DMA bytes (rel err ~5e-3 vs the
    2e-2 gate).
  * Input streams in [128,1024] halves so compute starts ~1.5 us in; output
    DMAs go through gpsimd (SWDGE) to keep the serialized HWDGE overhead
    (~630 ns/DMA) off the input path.

Device per core:
  in:  xT  [1024, 2048] bf16 — row g*128+p = feature, col = batch in shard.
  At   [128, 24*128] bf16: tiles 0-3: 4 lhsT blocks each; tiles 4-7: 2 each.
  coef [128, 24] f32: 0-7 r9, 8-15 bias, 16-19 s8 (tiles 4-7).
  out: outT [1024, 2048] bf16, same layout as xT.
"""
import numpy as np
import ml_dtypes

import concourse.mybir as mybir
import concourse.tile as tile
from concourse import bacc, bass_utils

F32 = mybir.dt.float32
BF16 = mybir.dt.bfloat16
MULT = mybir.AluOpType.mult
ADD = mybir.AluOpType.add

N_CORES = 8
BATCH = 16384
N = 1024
B_CORE = BATCH // N_CORES   # 2048
CHUNK = B_CORE              # single chunk
N_CHUNKS = 1
SUB = 512
N_SUBS = CHUNK // SUB       # 4

S7_PAIRS = [(0, 1), (2, 3), (4, 5), (6, 7)]
S8_PAIRS = [(0, 2), (1, 3), (4, 6), (5, 7)]
S9_PAIRS = [(0, 4), (1, 5), (2, 6), (3, 7)]
OTHER7 = {0: 1, 1: 0, 2: 3, 3: 2, 4: 5, 5: 4, 6: 7, 7: 6}
P8 = {0: 2, 2: 0, 1: 3, 3: 1, 4: 6, 6: 4, 5: 7, 7: 5}
P9 = {0: 4, 4: 0, 1: 5, 5: 1, 2: 6, 6: 2, 3: 7, 7: 3}
FOLDED = (0, 1, 2, 3)       # stage 8 folded into PE
VECT = (4, 5, 6, 7)         # stage 8 on DVE
SRC = {t: (t, OTHER7[t], P8[t], OTHER7[P8[t]]) for t in FOLDED}

_compiled = {}


def _woff(t, k):
    if t < 4:
        return (t * 4 + k) * 128
    return (16 + (t - 4) * 2 + k) * 128


def _emit_kernel(loop_reps=None):
    nc = bacc.Bacc("TRN2", target_bir_lowering=False, debug=False)
    xT = nc.dram_tensor("xT", [N, CHUNK], BF16, kind="ExternalInput").ap()
    At = nc.dram_tensor("At", [128, 24 * 128], BF16, kind="ExternalInput").ap()
    coef = nc.dram_tensor("coef", [128, 24], F32, kind="ExternalInput").ap()
    outT = nc.dram_tensor("outT", [N, CHUNK], BF16, kind="ExternalOutput").ap()

    with tile.TileContext(nc) as tc:
        with (
            tc.tile_pool(name="const", bufs=1) as cpool,
            tc.tile_pool(name="xin", bufs=8) as xpool,
            tc.tile_pool(name="uo", bufs=16) as upool,
            tc.tile_pool(name="v8o", bufs=8) as v8pool,
            tc.tile_pool(name="zo", bufs=8) as zpool,
            tc.tile_pool(name="v9o", bufs=16) as v9pool,
            tc.tile_pool(name="outb", bufs=8) as opool,
            tc.tile_pool(name="ps", bufs=8, space="PSUM") as ppool,
        ):
            at = cpool.tile([128, 24 * 128], BF16, tag="at")
            nc.sync.dma_start(at[:], At[:])
            cf = cpool.tile([128, 24], F32, tag="cf")
            nc.sync.dma_start(cf[:], coef[:])

            def c(col):
                return cf[:, col:col + 1]

            def w(t, k):
                off = _woff(t, k)
                return at[:, off:off + 128]

            ORD = (4, 5, 6, 7, 0, 1, 2, 3)  # VECT tiles first everywhere

            def body():
                xt = [None] * 8
                for g in range(8):
                    xt[g] = xpool.tile([128, CHUNK], BF16, tag="xt",
                                       name=f"xt{g}")
                # stream input in halves, VECT tiles first, so the 2-matmul
                # tiles and their evac/DVE chain start early
                for half in range(2):
                    hs = slice(half * 1024, (half + 1) * 1024)
                    for g in ORD:
                        nc.sync.dma_start(xt[g][:, hs],
                                          xT[g * 128:(g + 1) * 128, hs])
                ot = [None] * 8
                for g in range(8):
                    ot[g] = opool.tile([128, CHUNK], BF16, tag="ot",
                                       name=f"ot{g}")
                u = [None] * 8
                for sub in range(N_SUBS):
                    sl = slice(sub * SUB, (sub + 1) * SUB)
                    half = sub // 2
                    usl = slice((sub % 2) * SUB, (sub % 2) * SUB + SUB)
                    ps = [None] * 8
                    for t in ORD:
                        ps[t] = ppool.tile([128, SUB], F32, tag="ps",
                                           name=f"ps{t}")
                        if t in VECT:
                            nc.tensor.matmul(ps[t][:], w(t, 0), xt[t][:, sl],
                                             start=True, stop=False)
                            nc.tensor.matmul(ps[t][:], w(t, 1),
                                             xt[OTHER7[t]][:, sl],
                                             start=False, stop=True)
                        else:
                            for k in range(4):
                                nc.tensor.matmul(ps[t][:], w(t, k),
                                                 xt[SRC[t][k]][:, sl],
                                                 start=(k == 0), stop=(k == 3))
                    if sub % 2 == 0:
                        for t in range(8):
                            u[t] = upool.tile([128, 2 * SUB], BF16, tag="u",
                                              name=f"u{t}")
                    for t in ORD:
                        nc.scalar.copy(u[t][:, usl], ps[t][:])
                    if sub % 2 == 0:
                        continue
                    # a full [128,1024] half is evacuated: run the DVE chain
                    # at double width (amortizes per-op init, 4x/2x modes)
                    y = [u[t][:] for t in range(8)]
                    v8 = {}
                    for t in VECT:
                        v8[t] = v8pool.tile([128, 2 * SUB], BF16, tag="v8",
                                            name=f"v8{t}")
                        nc.vector.tensor_scalar(v8[t][:], u[P8[t]][:],
                                                c(16 + t - 4), None, op0=MULT)
                    for t in VECT:
                        z = zpool.tile([128, 2 * SUB], BF16, tag="z",
                                       name=f"z{t}")
                        nc.vector.tensor_tensor(z[:], u[t][:], v8[t][:],
                                                op=ADD)
                        y[t] = z[:]
                    # stage 9: prescale+bias then combine
                    v9 = [None] * 8
                    for t in (4, 5, 6, 7, 0, 1, 2, 3):
                        v9[t] = v9pool.tile([128, 2 * SUB], BF16, tag="v9",
                                            name=f"v9{t}")
                        nc.vector.tensor_scalar(v9[t][:], y[P9[t]],
                                                c(t), c(8 + t),
                                                op0=MULT, op1=ADD)
                    hsl = slice(half * 2 * SUB, (half + 1) * 2 * SUB)
                    for t in ORD:
                        nc.vector.tensor_tensor(ot[t][:, hsl], y[t],
                                                v9[t][:], op=ADD)
                    for t in ORD:
                        nc.sync.dma_start(
                            outT[t * 128:(t + 1) * 128, hsl],
                            ot[t][:, hsl])

            if loop_reps is not None:
                with tc.For_i(0, loop_reps, 1,
                              hint_engines=(mybir.EngineType.PE,
                                            mybir.EngineType.DVE,
                                            mybir.EngineType.Activation)):
                    body()
            else:
                body()

    nc.compile()
    return nc


def _get_compiled(loop_reps=None):
    if loop_reps not in _compiled:
        _compiled[loop_reps] = _emit_kernel(loop_reps)
    return _compiled[loop_reps]


def _build_A(twiddle):
    A = np.zeros((8, 128, 128), np.float64)
    for h in range(8):
        M = np.eye(128, dtype=np.float64)
        for idx in range(7):
            s = 1 << idx
            tw = twiddle[0, 0, idx].astype(np.float64).reshape(512 // s, s, 2, 2)
            tw_h = tw[h * (64 // s):(h + 1) * (64 // s)]
            Mv = M.reshape(64 // s, 2, s, 128)
            top, bot = Mv[:, 0], Mv[:, 1]
            M = np.stack(
                [tw_h[:, :, 0, 0][..., None] * top + tw_h[:, :, 0, 1][..., None] * bot,
                 tw_h[:, :, 1, 0][..., None] * top + tw_h[:, :, 1, 1][..., None] * bot],
                axis=1).reshape(128, 128)
        A[h] = M
    return A


def _coef_parts(twiddle):
    t8 = twiddle[0, 0, 8].reshape(2, 256, 2, 2).astype(np.float64)
    t9 = twiddle[0, 0, 9].reshape(512, 2, 2).astype(np.float64)
    c8d = np.zeros((8, 128)); c8o = np.zeros((8, 128))
    for gi, (p_, q_) in enumerate(S8_PAIRS):
        G, hp = divmod(gi, 2)
        cc = t8[G, hp * 128:(hp + 1) * 128]
        c8d[p_], c8o[p_] = cc[:, 0, 0], cc[:, 0, 1]
        c8d[q_], c8o[q_] = cc[:, 1, 1], cc[:, 1, 0]
    g = np.zeros((8, 128)); r9 = np.zeros((8, 128))
    for a, b in S9_PAIRS:
        e = t9[a * 128:(a + 1) * 128]
        g[a], g[b] = e[:, 0, 0], e[:, 1, 1]
        r9[a] = e[:, 0, 1] / e[:, 1, 1]
        r9[b] = e[:, 1, 0] / e[:, 0, 0]
    return c8d, c8o, g, r9


def _build_weights(twiddle):
    """At [128, 24*128] bf16 per _woff layout (lhsT blocks)."""
    A = _build_A(twiddle)
    t7 = twiddle[0, 0, 7].reshape(4, 128, 2, 2).astype(np.float64)
    B = np.zeros((8, 128, 128)); C = np.zeros((8, 128, 128))
    for gi, (p, q) in enumerate(S7_PAIRS):
        B[p] = np.diag(t7[gi, :, 0, 0]) @ A[p]
        C[p] = np.diag(t7[gi, :, 0, 1]) @ A[q]
        B[q] = np.diag(t7[gi, :, 1, 1]) @ A[q]
        C[q] = np.diag(t7[gi, :, 1, 0]) @ A[p]
    c8d, c8o, g, _ = _coef_parts(twiddle)
    At = np.zeros((128, 24 * 128), ml_dtypes.bfloat16)
    for t in FOLDED:
        pt = P8[t]
        Ws = [
            (g[t] * c8d[t])[:, None] * B[t],
            (g[t] * c8d[t])[:, None] * C[t],
            (g[t] * c8o[t])[:, None] * B[pt],
            (g[t] * c8o[t])[:, None] * C[pt],
        ]
        for k, W in enumerate(Ws):
            off = _woff(t, k)
            At[:, off:off + 128] = W.T.astype(ml_dtypes.bfloat16)
    for t in VECT:
        alpha = g[t] * c8d[t]
        for k, W in enumerate((alpha[:, None] * B[t], alpha[:, None] * C[t])):
            off = _woff(t, k)
            At[:, off:off + 128] = W.T.astype(ml_dtypes.bfloat16)
    return At


def _build_coef(twiddle, bias):
    c8d, c8o, g, r9 = _coef_parts(twiddle)
    coef = np.zeros((128, 24), np.float32)
    coef[:, 0:8] = r9.T
    coef[:, 8:16] = np.asarray(bias, np.float64).reshape(8, 128).T
    for t in VECT:
        alpha_p = g[P8[t]] * c8d[P8[t]]
        coef[:, 16 + t - 4] = g[t] * c8o[t] / alpha_p
    return coef


def _build_xT(shard):
    """shard [B_CORE, 1024] fp32 -> [1024, B_CORE] bf16 (transposed)."""
    return np.ascontiguousarray(shard.T).astype(ml_dtypes.bfloat16)


def kernel(input, twiddle, bias):
    input = np.asarray(input)
    twiddle = np.asarray(twiddle)
    bias = np.asarray(bias)
    nc = _get_compiled()

    At = _build_weights(twiddle)
    coef = _build_coef(twiddle, bias)
    in_maps = []
    for cid in range(N_CORES):
        shard = input[cid * B_CORE:(cid + 1) * B_CORE, :]
        in_maps.append({"xT": _build_xT(shard), "At": At, "coef": coef})

    res = bass_utils.run_bass_kernel_spmd(nc, in_maps,
                                          core_ids=list(range(N_CORES)))
    out = np.empty((BATCH, N), np.float32)
    for cid in range(N_CORES):
        o = res.results[cid]["outT"]  # [1024, B_CORE] bf16
        out[cid * B_CORE:(cid + 1) * B_CORE, :] = o.T.astype(np.float32)
    return out


# revision 10
# speedup vs baseline: 1.4995x; 1.0035x over previous
"""Trainium2 Bass kernel for nn_Butterfly_1580547970089.

Butterfly multiply (n=1024, log_n=10, nstacks=nblocks=1) + bias over a
16384-row batch, data-parallel across 8 NeuronCores (2048 rows each).

Decomposition (per core, features on partitions, batch on the free dim):
  * Stages 0-7 are composed on the host into dense 128x128 bf16 matrices.
    For out-tiles 0-3 stage 8 is ALSO folded into the weights (4 matmuls
    from sources {t, other7(t), P8(t), other7(P8(t))}); out-tiles 4-7 use
    2 matmuls and do stage 8 on DVE. This splits the load: PE ~20 us,
    DVE ~23 us, ACT ~23 us, DMA bus ~26 us (the floor).
  * The per-feature stage-9 bake g_t and tile-diagonal stage-8 coefficient
    are folded into the matmul weights, so vector stages use only fast-mode
    ops (scalar_tensor_tensor has NO DVE fast mode; tensor_scalar 4x and
    tensor_tensor 2x do):
      evac:    u_t = bf16(PSUM_t)                      (ACT, the fp32 pass)
      stage 8: v8_t = u_{P8(t)} * s8_t                 (ts 1-scalar, 4x)
               z_t  = u_t + v8_t                       (tt, 2x)   tiles 4-7
      stage 9: v9_t = y_{P9(t)} * r9_t + b_t           (ts2 on DVE 4x /
               out_t = y_t + v9_t                       ACT activation; tt 2x)
    where y_t = u_t (folded tiles) or z_t (vector tiles). Scalars are
    per-partition [128,1] fp32 APs; ratios are multiplicative (safe), bias
    adds at its natural place.
  * Single bf16 plane in + bf16 out halves DMA bytes (rel err ~5e-3 vs the
    2e-2 gate).
  * Input streams in [128,1024] halves so compute starts ~1.5 us in; output
    DMAs go through gpsimd (SWDGE) to keep the serialized HWDGE overhead
    (~630 ns/DMA) off the input path.

Device per core:
  in:  xT  [1024, 2048] bf16 — row g*128+p = feature, col = batch in shard.
  At   [128, 24*128] bf16: tiles 0-3: 4 lhsT blocks each; tiles 4-7: 2 each.
  coef [128, 24] f32: 0-7 r9, 8-15 bias, 16-19 s8 (tiles 4-7).
  out: outT [1024, 2048] bf16, same layout as xT.
"""
import numpy as np
import ml_dtypes

import concourse.mybir as mybir
import concourse.tile as tile
from concourse import bacc, bass_utils

F32 = mybir.dt.float32
BF16 = mybir.dt.bfloat16
MULT = mybir.AluOpType.mult
ADD = mybir.AluOpType.add

N_CORES = 8
BATCH = 16384
N = 1024
B_CORE = BATCH // N_CORES   # 2048
CHUNK = B_CORE              # single chunk
N_CHUNKS = 1
SUB = 512
N_SUBS = CHUNK // SUB       # 4

S7_PAIRS = [(0, 1), (2, 3), (4, 5), (6, 7)]
S8_PAIRS = [(0, 2), (1, 3), (4, 6), (5, 7)]
S9_PAIRS = [(0, 4), (1, 5), (2, 6), (3, 7)]
OTHER7 = {0: 1, 1: 0, 2: 3, 3: 2, 4: 5, 5: 4, 6: 7, 7: 6}
P8 = {0: 2, 2: 0, 1: 3, 3: 1, 4: 6, 6: 4, 5: 7, 7: 5}
P9 = {0: 4, 4: 0, 1: 5, 5: 1, 2: 6, 6: 2, 3: 7, 7: 3}
FOLDED = (0, 1, 2, 3, 4, 6)  # stage 8 folded into PE (4 matmuls)
VECT = (5, 7)                # stage 8 on DVE (2 matmuls)
SRC = {t: (t, OTHER7[t], P8[t], OTHER7[P8[t]]) for t in FOLDED}

_compiled = {}

_WOFF = {}
for _i, _t in enumerate(FOLDED):
    for _k in range(4):
        _WOFF[(_t, _k)] = (_i * 4 + _k) * 128
for _i, _t in enumerate(VECT):
    for _k in range(2):
        _WOFF[(_t, _k)] = (len(FOLDED) * 4 + _i * 2 + _k) * 128
N_WBLK = len(FOLDED) * 4 + len(VECT) * 2  # 28


def _woff(t, k):
    return _WOFF[(t, k)]


def _emit_kernel(loop_reps=None):
    nc = bacc.Bacc("TRN2", target_bir_lowering=False, debug=False)
    xT = nc.dram_tensor("xT", [N, CHUNK], BF16, kind="ExternalInput").ap()
    At = nc.dram_tensor("At", [128, N_WBLK * 128], BF16, kind="ExternalInput").ap()
    coef = nc.dram_tensor("coef", [128, 24], F32, kind="ExternalInput").ap()
    outT = nc.dram_tensor("outT", [N, CHUNK], BF16, kind="ExternalOutput").ap()

    with tile.TileContext(nc) as tc:
        with (
            tc.tile_pool(name="const", bufs=1) as cpool,
            tc.tile_pool(name="xin", bufs=8) as xpool,
            tc.tile_pool(name="uo", bufs=16) as upool,
            tc.tile_pool(name="v8o", bufs=8) as v8pool,
            tc.tile_pool(name="zo", bufs=8) as zpool,
            tc.tile_pool(name="v9o", bufs=16) as v9pool,
            tc.tile_pool(name="outb", bufs=8) as opool,
            tc.tile_pool(name="ps", bufs=8, space="PSUM") as ppool,
        ):
            at = cpool.tile([128, N_WBLK * 128], BF16, tag="at")
            nc.sync.dma_start(at[:], At[:])
            cf = cpool.tile([128, 24], F32, tag="cf")
            nc.sync.dma_start(cf[:], coef[:])

            def c(col):
                return cf[:, col:col + 1]

            def w(t, k):
                off = _woff(t, k)
                return at[:, off:off + 128]

            ORD = (4, 5, 6, 7, 0, 1, 2, 3)  # VECT tiles first everywhere

            def body():
                xt = [None] * 8
                for g in range(8):
                    xt[g] = xpool.tile([128, CHUNK], BF16, tag="xt",
                                       name=f"xt{g}")
                # stream input in halves, VECT tiles first, so the 2-matmul
                # tiles and their evac/DVE chain start early
                for half in range(2):
                    hs = slice(half * 1024, (half + 1) * 1024)
                    for g in ORD:
                        nc.sync.dma_start(xt[g][:, hs],
                                          xT[g * 128:(g + 1) * 128, hs])
                ot = [None] * 8
                for g in range(8):
                    ot[g] = opool.tile([128, CHUNK], BF16, tag="ot",
                                       name=f"ot{g}")
                for sub in range(N_SUBS):
                    sl = slice(sub * SUB, (sub + 1) * SUB)
                    ps = [None] * 8
                    for t in ORD:
                        ps[t] = ppool.tile([128, SUB], F32, tag="ps",
                                           name=f"ps{t}")
                        if t in VECT:
                            nc.tensor.matmul(ps[t][:], w(t, 0), xt[t][:, sl],
                                             start=True, stop=False)
                            nc.tensor.matmul(ps[t][:], w(t, 1),
                                             xt[OTHER7[t]][:, sl],
                                             start=False, stop=True)
                        else:
                            for k in range(4):
                                nc.tensor.matmul(ps[t][:], w(t, k),
                                                 xt[SRC[t][k]][:, sl],
                                                 start=(k == 0), stop=(k == 3))
                    u = [None] * 8
                    for t in (5, 7, 4, 6, 0, 1, 2, 3):  # VECT evacs first
                        u[t] = upool.tile([128, SUB], BF16, tag="u",
                                          name=f"u{t}")
                        nc.scalar.copy(u[t][:], ps[t][:])
                    y = [u[t][:] for t in range(8)]
                    v8 = {}
                    for t in VECT:
                        v8[t] = v8pool.tile([128, SUB], BF16, tag="v8",
                                            name=f"v8{t}")
                        nc.vector.tensor_scalar(v8[t][:], u[P8[t]][:],
                                                c(16 + VECT.index(t)), None,
                                                op0=MULT)
                    for t in VECT:
                        z = zpool.tile([128, SUB], BF16, tag="z",
                                       name=f"z{t}")
                        nc.vector.tensor_tensor(z[:], u[t][:], v8[t][:],
                                                op=ADD)
                        y[t] = z[:]
                    # stage 9: prescale+bias then combine
                    v9 = [None] * 8
                    for t in (5, 7, 0, 2, 4, 6, 1, 3):  # v9 of 1,3 need z
                        v9[t] = v9pool.tile([128, SUB], BF16, tag="v9",
                                            name=f"v9{t}")
                        nc.vector.tensor_scalar(v9[t][:], y[P9[t]],
                                                c(t), c(8 + t),
                                                op0=MULT, op1=ADD)
                    for t in ORD:
                        nc.vector.tensor_tensor(ot[t][:, sl], y[t],
                                                v9[t][:], op=ADD)
                    # output DMA per completed half
                    if sub % 2 == 1:
                        hsl = slice((sub - 1) * SUB, (sub + 1) * SUB)
                        for t in ORD:
                            nc.sync.dma_start(
                                outT[t * 128:(t + 1) * 128, hsl],
                                ot[t][:, hsl])

            if loop_reps is not None:
                with tc.For_i(0, loop_reps, 1,
                              hint_engines=(mybir.EngineType.PE,
                                            mybir.EngineType.DVE,
                                            mybir.EngineType.Activation)):
                    body()
            else:
                body()

    nc.compile()
    return nc


def _get_compiled(loop_reps=None):
    if loop_reps not in _compiled:
        _compiled[loop_reps] = _emit_kernel(loop_reps)
    return _compiled[loop_reps]


def _build_A(twiddle):
    A = np.zeros((8, 128, 128), np.float64)
    for h in range(8):
        M = np.eye(128, dtype=np.float64)
        for idx in range(7):
            s = 1 << idx
            tw = twiddle[0, 0, idx].astype(np.float64).reshape(512 // s, s, 2, 2)
            tw_h = tw[h * (64 // s):(h + 1) * (64 // s)]
            Mv = M.reshape(64 // s, 2, s, 128)
            top, bot = Mv[:, 0], Mv[:, 1]
            M = np.stack(
                [tw_h[:, :, 0, 0][..., None] * top + tw_h[:, :, 0, 1][..., None] * bot,
                 tw_h[:, :, 1, 0][..., None] * top + tw_h[:, :, 1, 1][..., None] * bot],
                axis=1).reshape(128, 128)
        A[h] = M
    return A


def _coef_parts(twiddle):
    t8 = twiddle[0, 0, 8].reshape(2, 256, 2, 2).astype(np.float64)
    t9 = twiddle[0, 0, 9].reshape(512, 2, 2).astype(np.float64)
    c8d = np.zeros((8, 128)); c8o = np.zeros((8, 128))
    for gi, (p_, q_) in enumerate(S8_PAIRS):
        G, hp = divmod(gi, 2)
        cc = t8[G, hp * 128:(hp + 1) * 128]
        c8d[p_], c8o[p_] = cc[:, 0, 0], cc[:, 0, 1]
        c8d[q_], c8o[q_] = cc[:, 1, 1], cc[:, 1, 0]
    g = np.zeros((8, 128)); r9 = np.zeros((8, 128))
    for a, b in S9_PAIRS:
        e = t9[a * 128:(a + 1) * 128]
        g[a], g[b] = e[:, 0, 0], e[:, 1, 1]
        r9[a] = e[:, 0, 1] / e[:, 1, 1]
        r9[b] = e[:, 1, 0] / e[:, 0, 0]
    return c8d, c8o, g, r9


def _build_weights(twiddle):
    """At [128, 24*128] bf16 per _woff layout (lhsT blocks)."""
    A = _build_A(twiddle)
    t7 = twiddle[0, 0, 7].reshape(4, 128, 2, 2).astype(np.float64)
    B = np.zeros((8, 128, 128)); C = np.zeros((8, 128, 128))
    for gi, (p, q) in enumerate(S7_PAIRS):
        B[p] = np.diag(t7[gi, :, 0, 0]) @ A[p]
        C[p] = np.diag(t7[gi, :, 0, 1]) @ A[q]
        B[q] = np.diag(t7[gi, :, 1, 1]) @ A[q]
        C[q] = np.diag(t7[gi, :, 1, 0]) @ A[p]
    c8d, c8o, g, _ = _coef_parts(twiddle)
    At = np.zeros((128, N_WBLK * 128), ml_dtypes.bfloat16)
    for t in FOLDED:
        pt = P8[t]
        Ws = [
            (g[t] * c8d[t])[:, None] * B[t],
            (g[t] * c8d[t])[:, None] * C[t],
            (g[t] * c8o[t])[:, None] * B[pt],
            (g[t] * c8o[t])[:, None] * C[pt],
        ]
        for k, W in enumerate(Ws):
            off = _woff(t, k)
            At[:, off:off + 128] = W.T.astype(ml_dtypes.bfloat16)
    for t in VECT:
        alpha = g[t] * c8d[t]
        for k, W in enumerate((alpha[:, None] * B[t], alpha[:, None] * C[t])):
            off = _woff(t, k)
            At[:, off:off + 128] = W.T.astype(ml_dtypes.bfloat16)
    return At


def _build_coef(twiddle, bias):
    c8d, c8o, g, r9 = _coef_parts(twiddle)
    coef = np.zeros((128, 24), np.float32)
    coef[:, 0:8] = r9.T
    coef[:, 8:16] = np.asarray(bias, np.float64).reshape(8, 128).T
    for i, t in enumerate(VECT):
        alpha_p = g[P8[t]] * c8d[P8[t]]
        coef[:, 16 + i] = g[t] * c8o[t] / alpha_p
    return coef


def _build_xT(shard):
    """shard [B_CORE, 1024] fp32 -> [1024, B_CORE] bf16 (transposed)."""
    return np.ascontiguousarray(shard.T).astype(ml_dtypes.bfloat16)


def kernel(input, twiddle, bias):
    input = np.asarray(input)
    twiddle = np.asarray(twiddle)
    bias = np.asarray(bias)
    nc = _get_compiled()

    At = _build_weights(twiddle)
    coef = _build_coef(twiddle, bias)
    in_maps = []
    for cid in range(N_CORES):
        shard = input[cid * B_CORE:(cid + 1) * B_CORE, :]
        in_maps.append({"xT": _build_xT(shard), "At": At, "coef": coef})

    res = bass_utils.run_bass_kernel_spmd(nc, in_maps,
                                          core_ids=list(range(N_CORES)))
    out = np.empty((BATCH, N), np.float32)
    for cid in range(N_CORES):
        o = res.results[cid]["outT"]  # [1024, B_CORE] bf16
        out[cid * B_CORE:(cid + 1) * B_CORE, :] = o.T.astype(np.float32)
    return out


# revision 12
# speedup vs baseline: 1.5991x; 1.0664x over previous
"""Trainium2 Bass kernel for nn_Butterfly_1580547970089.

Butterfly multiply (n=1024, log_n=10, nstacks=nblocks=1) + bias over a
16384-row batch, data-parallel across 8 NeuronCores (2048 rows each).

Decomposition (per core, features on partitions, batch on the free dim):
  * Stages 0-7 are composed on the host into dense 128x128 bf16 matrices.
    For out-tiles 0-3 stage 8 is ALSO folded into the weights (4 matmuls
    from sources {t, other7(t), P8(t), other7(P8(t))}); out-tiles 4-7 use
    2 matmuls and do stage 8 on DVE. This splits the load: PE ~20 us,
    DVE ~23 us, ACT ~23 us, DMA bus ~26 us (the floor).
  * The per-feature stage-9 bake g_t and tile-diagonal stage-8 coefficient
    are folded into the matmul weights, so vector stages use only fast-mode
    ops (scalar_tensor_tensor has NO DVE fast mode; tensor_scalar 4x and
    tensor_tensor 2x do):
      evac:    u_t = bf16(PSUM_t)                      (ACT, the fp32 pass)
      stage 8: v8_t = u_{P8(t)} * s8_t                 (ts 1-scalar, 4x)
               z_t  = u_t + v8_t                       (tt, 2x)   tiles 4-7
      stage 9: v9_t = y_{P9(t)} * r9_t + b_t           (ts2 on DVE 4x /
               out_t = y_t + v9_t                       ACT activation; tt 2x)
    where y_t = u_t (folded tiles) or z_t (vector tiles). Scalars are
    per-partition [128,1] fp32 APs; ratios are multiplicative (safe), bias
    adds at its natural place.
  * Single bf16 plane in + bf16 out halves DMA bytes (rel err ~5e-3 vs the
    2e-2 gate).
  * Input streams in [128,1024] halves so compute starts ~1.5 us in; output
    DMAs go through gpsimd (SWDGE) to keep the serialized HWDGE overhead
    (~630 ns/DMA) off the input path.

Device per core:
  in:  xT  [1024, 2048] bf16 — row g*128+p = feature, col = batch in shard.
  At   [128, 24*128] bf16: tiles 0-3: 4 lhsT blocks each; tiles 4-7: 2 each.
  coef [128, 24] f32: 0-7 r9, 8-15 bias, 16-19 s8 (tiles 4-7).
  out: outT [1024, 2048] bf16, same layout as xT.
"""
import numpy as np
import ml_dtypes

import concourse.mybir as mybir
import concourse.tile as tile
from concourse import bacc, bass_utils

F32 = mybir.dt.float32
BF16 = mybir.dt.bfloat16
MULT = mybir.AluOpType.mult
ADD = mybir.AluOpType.add

N_CORES = 8
BATCH = 16384
N = 1024
B_CORE = BATCH // N_CORES   # 2048
CHUNK = B_CORE              # single chunk
N_CHUNKS = 1
SUB = 512
N_SUBS = CHUNK // SUB       # 4

S7_PAIRS = [(0, 1), (2, 3), (4, 5), (6, 7)]
S8_PAIRS = [(0, 2), (1, 3), (4, 6), (5, 7)]
S9_PAIRS = [(0, 4), (1, 5), (2, 6), (3, 7)]
OTHER7 = {0: 1, 1: 0, 2: 3, 3: 2, 4: 5, 5: 4, 6: 7, 7: 6}
P8 = {0: 2, 2: 0, 1: 3, 3: 1, 4: 6, 6: 4, 5: 7, 7: 5}
P9 = {0: 4, 4: 0, 1: 5, 5: 1, 2: 6, 6: 2, 3: 7, 7: 3}
FOLDED = (0, 1, 2, 3, 4, 6)  # stage 8 folded into PE (4 matmuls)
VECT = (5, 7)                # stage 8 on DVE (2 matmuls)
SRC = {t: (t, OTHER7[t], P8[t], OTHER7[P8[t]]) for t in FOLDED}

_compiled = {}

_WOFF = {}
for _i, _t in enumerate(FOLDED):
    for _k in range(4):
        _WOFF[(_t, _k)] = (_i * 4 + _k) * 128
for _i, _t in enumerate(VECT):
    for _k in range(2):
        _WOFF[(_t, _k)] = (len(FOLDED) * 4 + _i * 2 + _k) * 128
N_WBLK = len(FOLDED) * 4 + len(VECT) * 2  # 28


def _woff(t, k):
    return _WOFF[(t, k)]


def _emit_kernel(loop_reps=None):
    nc = bacc.Bacc("TRN2", target_bir_lowering=False, debug=False)
    xT = nc.dram_tensor("xT", [N, CHUNK], BF16, kind="ExternalInput").ap()
    At = nc.dram_tensor("At", [128, N_WBLK * 128], BF16, kind="ExternalInput").ap()
    coef = nc.dram_tensor("coef", [128, 24], F32, kind="ExternalInput").ap()
    outT = nc.dram_tensor("outT", [N, CHUNK], BF16, kind="ExternalOutput").ap()

    with tile.TileContext(nc) as tc:
        with (
            tc.tile_pool(name="const", bufs=1) as cpool,
            tc.tile_pool(name="xin", bufs=8) as xpool,
            tc.tile_pool(name="uo", bufs=16) as upool,
            tc.tile_pool(name="v8o", bufs=8) as v8pool,
            tc.tile_pool(name="zo", bufs=8) as zpool,
            tc.tile_pool(name="v9o", bufs=16) as v9pool,
            tc.tile_pool(name="outb", bufs=8) as opool,
            tc.tile_pool(name="ps", bufs=8, space="PSUM") as ppool,
        ):
            at = cpool.tile([128, N_WBLK * 128], BF16, tag="at")
            nc.sync.dma_start(at[:], At[:])
            cf = cpool.tile([128, 24], F32, tag="cf")
            nc.sync.dma_start(cf[:], coef[:])

            def c(col):
                return cf[:, col:col + 1]

            def w(t, k):
                off = _woff(t, k)
                return at[:, off:off + 128]

            ORD = (4, 5, 6, 7, 0, 1, 2, 3)  # VECT tiles first everywhere

            def body():
                xt = [None] * 8
                for g in range(8):
                    xt[g] = xpool.tile([128, CHUNK], BF16, tag="xt",
                                       name=f"xt{g}")
                # stream input: sub-0 slices of all tiles first (compute
                # starts ~3 us in), then the remainder per tile
                for g in ORD:
                    nc.sync.dma_start(xt[g][:, 0:SUB],
                                      xT[g * 128:(g + 1) * 128, 0:SUB])
                for g in ORD:
                    nc.sync.dma_start(xt[g][:, SUB:CHUNK],
                                      xT[g * 128:(g + 1) * 128, SUB:CHUNK])
                ot = [None] * 8
                for g in range(8):
                    ot[g] = opool.tile([128, CHUNK], BF16, tag="ot",
                                       name=f"ot{g}")
                for sub in range(N_SUBS):
                    sl = slice(sub * SUB, (sub + 1) * SUB)
                    ps = [None] * 8
                    for t in ORD:
                        ps[t] = ppool.tile([128, SUB], F32, tag="ps",
                                           name=f"ps{t}")
                        if t in VECT:
                            nc.tensor.matmul(ps[t][:], w(t, 0), xt[t][:, sl],
                                             start=True, stop=False)
                            nc.tensor.matmul(ps[t][:], w(t, 1),
                                             xt[OTHER7[t]][:, sl],
                                             start=False, stop=True)
                        else:
                            for k in range(4):
                                nc.tensor.matmul(ps[t][:], w(t, k),
                                                 xt[SRC[t][k]][:, sl],
                                                 start=(k == 0), stop=(k == 3))
                    u = [None] * 8
                    for t in (5, 7, 4, 6, 0, 1, 2, 3):  # VECT evacs first
                        u[t] = upool.tile([128, SUB], BF16, tag="u",
                                          name=f"u{t}")
                        nc.scalar.copy(u[t][:], ps[t][:])
                    y = [u[t][:] for t in range(8)]
                    v8 = {}
                    for t in VECT:
                        v8[t] = v8pool.tile([128, SUB], BF16, tag="v8",
                                            name=f"v8{t}")
                        nc.vector.tensor_scalar(v8[t][:], u[P8[t]][:],
                                                c(16 + VECT.index(t)), None,
                                                op0=MULT)
                    for t in VECT:
                        z = zpool.tile([128, SUB], BF16, tag="z",
                                       name=f"z{t}")
                        nc.vector.tensor_tensor(z[:], u[t][:], v8[t][:],
                                                op=ADD)
                        y[t] = z[:]
                    # stage 9: prescale+bias then combine
                    v9 = [None] * 8
                    for t in (5, 7, 0, 2, 4, 6, 1, 3):  # v9 of 1,3 need z
                        v9[t] = v9pool.tile([128, SUB], BF16, tag="v9",
                                            name=f"v9{t}")
                        nc.vector.tensor_scalar(v9[t][:], y[P9[t]],
                                                c(t), c(8 + t),
                                                op0=MULT, op1=ADD)
                    for t in ORD:
                        nc.vector.tensor_tensor(ot[t][:, sl], y[t],
                                                v9[t][:], op=ADD)
                    # output DMA: [0:1024] after sub 1, then per-sub slices
                    # (finer at the end to shorten the tail)
                    if sub >= 1:
                        lo = 0 if sub == 1 else sub * SUB
                        osl = slice(lo, (sub + 1) * SUB)
                        for t in ORD:
                            nc.sync.dma_start(
                                outT[t * 128:(t + 1) * 128, osl],
                                ot[t][:, osl])

            if loop_reps is not None:
                with tc.For_i(0, loop_reps, 1,
                              hint_engines=(mybir.EngineType.PE,
                                            mybir.EngineType.DVE,
                                            mybir.EngineType.Activation)):
                    body()
            else:
                body()

    nc.compile()
    return nc


def _get_compiled(loop_reps=None):
    if loop_reps not in _compiled:
        _compiled[loop_reps] = _emit_kernel(loop_reps)
    return _compiled[loop_reps]


def _build_A(twiddle):
    A = np.zeros((8, 128, 128), np.float64)
    for h in range(8):
        M = np.eye(128, dtype=np.float64)
        for idx in range(7):
            s = 1 << idx
            tw = twiddle[0, 0, idx].astype(np.float64).reshape(512 // s, s, 2, 2)
            tw_h = tw[h * (64 // s):(h + 1) * (64 // s)]
            Mv = M.reshape(64 // s, 2, s, 128)
            top, bot = Mv[:, 0], Mv[:, 1]
            M = np.stack(
                [tw_h[:, :, 0, 0][..., None] * top + tw_h[:, :, 0, 1][..., None] * bot,
                 tw_h[:, :, 1, 0][..., None] * top + tw_h[:, :, 1, 1][..., None] * bot],
                axis=1).reshape(128, 128)
        A[h] = M
    return A


def _coef_parts(twiddle):
    t8 = twiddle[0, 0, 8].reshape(2, 256, 2, 2).astype(np.float64)
    t9 = twiddle[0, 0, 9].reshape(512, 2, 2).astype(np.float64)
    c8d = np.zeros((8, 128)); c8o = np.zeros((8, 128))
    for gi, (p_, q_) in enumerate(S8_PAIRS):
        G, hp = divmod(gi, 2)
        cc = t8[G, hp * 128:(hp + 1) * 128]
        c8d[p_], c8o[p_] = cc[:, 0, 0], cc[:, 0, 1]
        c8d[q_], c8o[q_] = cc[:, 1, 1], cc[:, 1, 0]
    g = np.zeros((8, 128)); r9 = np.zeros((8, 128))
    for a, b in S9_PAIRS:
        e = t9[a * 128:(a + 1) * 128]
        g[a], g[b] = e[:, 0, 0], e[:, 1, 1]
        r9[a] = e[:, 0, 1] / e[:, 1, 1]
        r9[b] = e[:, 1, 0] / e[:, 0, 0]
    return c8d, c8o, g, r9


def _build_weights(twiddle):
    """At [128, 24*128] bf16 per _woff layout (lhsT blocks)."""
    A = _build_A(twiddle)
    t7 = twiddle[0, 0, 7].reshape(4, 128, 2, 2).astype(np.float64)
    B = np.zeros((8, 128, 128)); C = np.zeros((8, 128, 128))
    for gi, (p, q) in enumerate(S7_PAIRS):
        B[p] = np.diag(t7[gi, :, 0, 0]) @ A[p]
        C[p] = np.diag(t7[gi, :, 0, 1]) @ A[q]
        B[q] = np.diag(t7[gi, :, 1, 1]) @ A[q]
        C[q] = np.diag(t7[gi, :, 1, 0]) @ A[p]
    c8d, c8o, g, _ = _coef_parts(twiddle)
    At = np.zeros((128, N_WBLK * 128), ml_dtypes.bfloat16)
    for t in FOLDED:
        pt = P8[t]
        Ws = [
            (g[t] * c8d[t])[:, None] * B[t],
            (g[t] * c8d[t])[:, None] * C[t],
            (g[t] * c8o[t])[:, None] * B[pt],
            (g[t] * c8o[t])[:, None] * C[pt],
        ]
        for k, W in enumerate(Ws):
            off = _woff(t, k)
            At[:, off:off + 128] = W.T.astype(ml_dtypes.bfloat16)
    for t in VECT:
        alpha = g[t] * c8d[t]
        for k, W in enumerate((alpha[:, None] * B[t], alpha[:, None] * C[t])):
            off = _woff(t, k)
            At[:, off:off + 128] = W.T.astype(ml_dtypes.bfloat16)
    return At


def _build_coef(twiddle, bias):
    c8d, c8o, g, r9 = _coef_parts(twiddle)
    coef = np.zeros((128, 24), np.float32)
    coef[:, 0:8] = r9.T
    coef[:, 8:16] = np.asarray(bias, np.float64).reshape(8, 128).T
    for i, t in enumerate(VECT):
        alpha_p = g[P8[t]] * c8d[P8[t]]
        coef[:, 16 + i] = g[t] * c8o[t] / alpha_p
    return coef


def _build_xT(shard):
    """shard [B_CORE, 1024] fp32 -> [1024, B_CORE] bf16 (transposed)."""
    return np.ascontiguousarray(shard.T).astype(ml_dtypes.bfloat16)


def kernel(input, twiddle, bias):
    input = np.asarray(input)
    twiddle = np.asarray(twiddle)
    bias = np.asarray(bias)
    nc = _get_compiled()

    At = _build_weights(twiddle)
    coef = _build_coef(twiddle, bias)
    in_maps = []
    for cid in range(N_CORES):
        shard = input[cid * B_CORE:(cid + 1) * B_CORE, :]
        in_maps.append({"xT": _build_xT(shard), "At": At, "coef": coef})

    res = bass_utils.run_bass_kernel_spmd(nc, in_maps,
                                          core_ids=list(range(N_CORES)))
    out = np.empty((BATCH, N), np.float32)
    for cid in range(N_CORES):
        o = res.results[cid]["outT"]  # [1024, B_CORE] bf16
        out[cid * B_CORE:(cid + 1) * B_CORE, :] = o.T.astype(np.float32)
    return out
